# revision 5
# baseline (speedup 1.0000x reference)
"""Trainium2 Bass kernel: multi-head attention (B=2, T=2048, E=1024, H=8, D=512),
bias-free QKV/O projections + RoPE + causal softmax.

Sharding: head-parallel across 8 NeuronCores. Core h computes head h fully:
  qT/kT = RoPE(Wq_h @ x.T), v = x @ Wv_h.T         (projection phase)
  scoresT[k,q] = kT.T @ qT   (per 512-wide q tile, causal-skipped k chunks)
  probsT = exp(scale*scoresT + mask)               (no max-subtraction: |s|<=9)
  attnT[d,q] = v.T @ probsT; rowsum accumulated on the Pool engine
  (elementwise adds of exp chunks) + ONE ones-matmul per q tile
  out_h = (attnT/rowsum).T @ Wo_h.T                (partial o_proj, [4096,1024])
Host sums the 8 partial outputs (equivalent to the all-reduce after o_proj).

Matmuls run fp32r except PV + o_proj which run bf16 (probs/v/attn/Wo are
bf16: probs in [0,1] and normalized attn are tolerant; PSUM accumulates fp32).
PSUM is managed as 8 explicitly-tagged banks so phase-to-phase reuse pairs
earliest-freed banks with earliest-needed, keeping the PE stream gap-free.
"""
from contextlib import ExitStack

import numpy as np

B, T, E, H, D = 2, 2048, 1024, 8, 512
NTOK = B * T
SCALE = float(1.0 / np.sqrt(D))
NEG = -1.0e30
ROPE_BASE = 10000.0

PROFILE = False          # set True (e.g. from test.py) to trace core 0
LAST_RESULTS = None      # BassKernelResults of the last run when PROFILE

_CACHE = {}


def _build():
    import concourse.tile as tile
    from concourse import bacc, mybir

    f32 = mybir.dt.float32
    f32r = mybir.dt.float32r
    bf16 = mybir.dt.bfloat16
    AF = mybir.ActivationFunctionType

    nc = bacc.Bacc("TRN2", target_bir_lowering=False, debug=False,
                   enable_asserts=False, num_devices=8)
    xT_d = nc.dram_tensor("xT", [E, NTOK], f32r, kind="ExternalInput").ap()
    wqT_d = nc.dram_tensor("wqT", [E, D], f32r, kind="ExternalInput").ap()
    wkT_d = nc.dram_tensor("wkT", [E, D], f32r, kind="ExternalInput").ap()
    wvT_d = nc.dram_tensor("wvT", [E, D], f32r, kind="ExternalInput").ap()
    woT_d = nc.dram_tensor("woT", [D, E], bf16, kind="ExternalInput").ap()
    cos_d = nc.dram_tensor("cosdt", [D // 2, T], f32, kind="ExternalInput").ap()
    sin_d = nc.dram_tensor("sindt", [D // 2, T], f32, kind="ExternalInput").ap()
    msk_d = nc.dram_tensor("mask4", [4, 128, 512], bf16, kind="ExternalInput").ap()
    out_d = nc.dram_tensor("out", [NTOK, E], f32, kind="ExternalOutput").ap()

    xT_r = xT_d.rearrange("(eo p) t -> p eo t", p=128)     # [128, 8, 4096]
    cos_r = cos_d.rearrange("(fo p) t -> p fo t", p=128)   # [128, 2, 2048]
    sin_r = sin_d.rearrange("(fo p) t -> p fo t", p=128)
    wq_v = wqT_d.rearrange("(eo p) d -> p eo d", p=128)
    wk_v = wkT_d.rearrange("(eo p) d -> p eo d", p=128)
    wv_v = wvT_d.rearrange("(eo p) d -> p eo d", p=128)

    with tile.TileContext(nc) as tc, ExitStack() as top:
        # ---- all pools top-level: no mid-kernel pool boundaries ----
        wp = top.enter_context(tc.tile_pool(name="wp", bufs=1))
        qkvp = top.enter_context(tc.tile_pool(name="qkvp", bufs=1))
        xp = top.enter_context(tc.tile_pool(name="xp", bufs=2))
        csp = top.enter_context(tc.tile_pool(name="csp", bufs=1))
        scrp = top.enter_context(tc.tile_pool(name="scrp", bufs=2))
        epp = top.enter_context(tc.tile_pool(name="epp", bufs=4))
        atp = top.enter_context(tc.tile_pool(name="atp", bufs=1))
        accp = top.enter_context(tc.tile_pool(name="accp", bufs=1))
        ivp = top.enter_context(tc.tile_pool(name="ivp", bufs=1))
        pbp = top.enter_context(tc.tile_pool(name="pbp", bufs=1, space="PSUM"))

        def PB(k):
            return pbp.tile([128, 512], f32, tag=f"B{k}", name=f"B{k}")

        wq_t = wp.tile([128, 8, D], f32r, tag="wq", name="wq")
        wk_t = wp.tile([128, 8, D], f32r, tag="wk", name="wk")
        wv_t = wp.tile([128, 8, D], f32r, tag="wv", name="wv")
        wv = [wv_t[:, e] for e in range(8)]
        mks = wp.tile([128, 4, 512], bf16, tag="mks", name="mks")
        mk = [mks[:, r] for r in range(4)]
        ones = wp.tile([128, 128], f32r, tag="ones", name="ones")
        expre = wp.tile([128, 1], f32, tag="expre", name="expre")
        wo_t = wp.tile([128, 4, E], bf16, tag="wo", name="wo")
        wo = [wo_t[:, d] for d in range(4)]

        qT = [qkvp.tile([128, T], f32r, tag=f"qT{d}", name=f"qT{d}") for d in range(4)]
        kT = [qkvp.tile([128, T], f32r, tag=f"kT{d}", name=f"kT{d}") for d in range(4)]
        vv = [qkvp.tile([128, D], bf16, tag=f"v{t}", name=f"v{t}") for t in range(16)]

        # ---- warmup: ramp the PE clock while startup DMAs stream in ----
        # (memset cannot target f32r: set an f32 scratch then cast-copy)
        onef = scrp.tile([128, 1024], f32, tag="scr", name="scr")
        nc.vector.memset(onef[:, :128], 1.0)
        nc.vector.tensor_copy(ones[:], onef[:, :128])
        warm_ps = PB(4)
        for w in range(10):
            nc.tensor.matmul(warm_ps[:, :128], ones[:], ones[:],
                             start=(w == 0), stop=(w == 9))
        # touch Exp so its ACT table set loads before the first score tile
        nc.scalar.activation(expre[:], warm_ps[:, :1], AF.Exp, scale=0.001)
        nc.vector.tensor_copy(expre[:], expre[:])

        def rope(dstT, i, j, fo, pi, pj, cs, sn, s0):
            c_, s_ = cs[:, fo], sn[:, fo]
            sA = scrp.tile([128, 1024], f32, tag="scr", name="scr")
            sB = scrp.tile([128, 1024], f32, tag="scr", name="scr")
            t0, t1 = sA[:, 0:512], sA[:, 512:1024]
            t2, t3 = sB[:, 0:512], sB[:, 512:1024]
            nc.vector.tensor_mul(t0[:], pi[:], c_)
            nc.vector.tensor_mul(t1[:], pj[:], s_)
            nc.vector.tensor_sub(dstT[i][:, s0:s0 + 512], t0[:], t1[:])
            nc.vector.tensor_mul(t2[:], pi[:], s_)
            nc.vector.tensor_mul(t3[:], pj[:], c_)
            nc.vector.tensor_add(dstT[j][:, s0:s0 + 512], t2[:], t3[:])

        for b in range(B):
            tok0 = b * T

            # ----- projection phase: qT/kT (RoPE'd) and v -----
            for tt in range(4):
                g0 = tok0 + tt * 512
                s0 = tt * 512
                xt = xp.tile([128, 8, 512], f32r, tag="xt", name="xt")
                nc.sync.dma_start(xt[:], xT_r[:, :, g0:g0 + 512])
                cs = csp.tile([128, 2, 512], f32, tag="cs", name="cs")
                sn = csp.tile([128, 2, 512], f32, tag="sn", name="sn")
                if b == 0 and tt == 0:
                    nc.sync.dma_start(
                        wv_t[:], wv_v)
                    nc.sync.dma_start(
                        wq_t[:], wq_v)
                    nc.sync.dma_start(cs[:], cos_r[:, :, s0:s0 + 512])
                    nc.sync.dma_start(sn[:], sin_r[:, :, s0:s0 + 512])
                    nc.sync.dma_start(
                        wk_t[:], wk_v)
                else:
                    nc.sync.dma_start(cs[:], cos_r[:, :, s0:s0 + 512])
                    nc.sync.dma_start(sn[:], sin_r[:, :, s0:s0 + 512])

                def emit_v(tt=tt):
                    for t4 in range(4):
                        ps_t = PB(2 + (t4 % 2))
                        for e in range(8):
                            nc.tensor.matmul(
                                ps_t[:],
                                xt[:, e, t4 * 128:(t4 + 1) * 128],
                                wv[e][:],
                                start=(e == 0), stop=(e == 7))
                        nc.scalar.copy(vv[tt * 4 + t4][:], ps_t[:])

                def emit_qk(s0=s0):
                    rot = (4, 5, 6, 7, 0, 1)
                    ri = 0
                    for w_t, dstT in ((wq_t, qT), (wk_t, kT)):
                        for i, j, fo in ((0, 2, 0), (1, 3, 1)):
                            ps2 = []
                            for dc in (i, j):
                                ps_t = PB(rot[ri % 6])
                                ri += 1
                                for e in range(8):
                                    nc.tensor.matmul(
                                        ps_t[:],
                                        w_t[:, e, dc * 128:(dc + 1) * 128],
                                        xt[:, e],
                                        start=(e == 0), stop=(e == 7))
                                ps2.append(ps_t)
                            rope(dstT, i, j, fo, ps2[0], ps2[1], cs, sn, s0)

                # v first (its ACT-copy evacuation has no cos/sin dependency)
                # except on the last token tile, where qk-first lets the P
                # phase end with a short ACT tail instead of a long DVE tail.
                if tt < 3:
                    emit_v()
                    emit_qk()
                else:
                    emit_qk()
                    emit_v()

            # ----- attention + o_proj phase -----
            SC = (0, 1, 2)
            ATB = (3, 4, 6, 7)
            if b == 0:
                nc.sync.dma_start(mks[:], msk_d.rearrange("r p q -> p r q"))
                nc.sync.dma_start(
                    wo_t[:], woT_d.rearrange("(do p) e -> p do e", p=128))

            sci = [0]

            def emit_oproj(n):
                q0 = n * 512
                for t4 in range(4):
                    ob = scrp.tile([128, 1024], f32, tag="scr", name="scr")
                    for et in range(2):
                        op_ps = PB(SC[sci[0] % 3])
                        sci[0] += 1
                        for dc in range(4):
                            nc.tensor.matmul(
                                op_ps[:],
                                at_sb[n % 2][dc][:, t4 * 128:(t4 + 1) * 128],
                                wo[dc][:, et * 512:(et + 1) * 512],
                                start=(dc == 0), stop=(dc == 3))
                        nc.scalar.copy(ob[:, et * 512:(et + 1) * 512], op_ps[:])
                    r0 = tok0 + q0 + t4 * 128
                    nc.sync.dma_start(out_d[r0:r0 + 128, :], ob[:])

            at_sb = {0: None, 1: None}
            for n in range(4):
                q0 = n * 512
                nch = 4 * n + 4
                attn_ps = [PB(ATB[d]) for d in range(4)]
                acc = accp.tile([128, 512], f32r, tag="acc", name="acc")

                def emit_pv(pex, pc, nch=nch, attn_ps=attn_ps):
                    for dc in range(4):
                        nc.tensor.matmul(
                            attn_ps[dc][:],
                            vv[pc][:, dc * 128:(dc + 1) * 128], pex[:],
                            start=(pc == 0), stop=(pc == nch - 1))

                pending = []
                oproj_done = (n == 0)
                for c in range(nch):
                    sc_ps = PB(SC[sci[0] % 3])
                    sci[0] += 1
                    for dc in range(4):
                        nc.tensor.matmul(
                            sc_ps[:],
                            kT[dc][:, c * 128:(c + 1) * 128],
                            qT[dc][:, q0:q0 + 512],
                            start=(dc == 0), stop=(dc == 3))
                    if c >= 4 * n:
                        nc.vector.tensor_add(sc_ps[:], sc_ps[:], mk[c - 4 * n][:])
                    ex = epp.tile([128, 512], bf16, tag="ex", name="ex")
                    nc.scalar.activation(ex[:], sc_ps[:], AF.Exp, scale=SCALE)
                    # rowsum accumulates on the (otherwise idle) Pool engine
                    if c == 0:
                        nc.gpsimd.tensor_copy(acc[:], ex[:])
                    else:
                        nc.gpsimd.tensor_add(acc[:], acc[:], ex[:])
                    pending.append((ex, c))
                    if len(pending) > 3:
                        emit_pv(*pending.pop(0))
                    if c == 2 and not oproj_done:
                        emit_oproj(n - 1)
                        oproj_done = True
                for pex, pc in pending:
                    emit_pv(pex, pc)
                # single partition-reduce matmul per q tile (broadcast rows)
                rs_ps = PB(5)
                nc.tensor.matmul(rs_ps[:], ones[:], acc[:], start=True, stop=True)
                inv = ivp.tile([128, 512], f32, tag="inv", name="inv")
                nc.vector.reciprocal(inv[:], rs_ps[:])
                at_sb[n % 2] = [
                    atp.tile([128, 512], bf16, tag=f"at{n % 2}_{dc}",
                             name=f"at{n % 2}_{dc}")
                    for dc in range(4)]
                for dc in range(4):
                    nc.vector.tensor_mul(
                        at_sb[n % 2][dc][:], attn_ps[dc][:], inv[:])
            emit_oproj(3)
    nc.compile()
    return nc


def _host_tables():
    import ml_dtypes
    inv_freq = 1.0 / (ROPE_BASE ** (np.arange(0, D, 2, dtype=np.float64) / D))
    ang = np.arange(T, dtype=np.float64)[:, None] * inv_freq[None, :]  # [T, D/2]
    cosdt = np.ascontiguousarray(np.cos(ang).T.astype(np.float32))     # [D/2, T]
    sindt = np.ascontiguousarray(np.sin(ang).T.astype(np.float32))
    mask4 = np.zeros((4, 128, 512), dtype=np.float32)
    kk = np.arange(128)[:, None]
    qq = np.arange(512)[None, :]
    for r in range(4):
        mask4[r] = np.where(128 * r + kk <= qq, 0.0, NEG).astype(np.float32)
    return cosdt, sindt, mask4.astype(ml_dtypes.bfloat16)


def kernel(x, Wq, Wk, Wv, Wo):
    global LAST_RESULTS
    import ml_dtypes
    from concourse import bass_utils

    if "nc" not in _CACHE:
        _CACHE["nc"] = _build()
    nc = _CACHE["nc"]

    x = np.asarray(x, dtype=np.float32)
    Wq = np.asarray(Wq, dtype=np.float32)
    Wk = np.asarray(Wk, dtype=np.float32)
    Wv = np.asarray(Wv, dtype=np.float32)
    Wo = np.asarray(Wo, dtype=np.float32)

    xT = np.ascontiguousarray(x.reshape(NTOK, E).T)          # [E, NTOK]
    cosdt, sindt, mask4 = _host_tables()

    in_maps = []
    for h in range(H):
        in_maps.append({
            "xT": xT,
            "wqT": np.ascontiguousarray(Wq[h * D:(h + 1) * D, :].T),
            "wkT": np.ascontiguousarray(Wk[h * D:(h + 1) * D, :].T),
            "wvT": np.ascontiguousarray(Wv[h * D:(h + 1) * D, :].T),
            "woT": np.ascontiguousarray(
                Wo[:, h * D:(h + 1) * D].T).astype(ml_dtypes.bfloat16),
            "cosdt": cosdt,
            "sindt": sindt,
            "mask4": mask4,
        })

    kwargs = {}
    if PROFILE:
        import sys
        import types
        import trn_agent_boot.trn_boot as _tb
        hook = _tb._ntff_profile_via_ctypes("/opt/axon/libaxon_pjrt.so")
        mod = types.ModuleType("antenv.axon_hooks")
        mod.get_axon_ntff_profile_hook = lambda: hook
        mod.set_axon_ntff_profile_hook = lambda h_: None
        sys.modules["antenv.axon_hooks"] = mod
        bass_utils.upload_artifacts = lambda tmpdir: tmpdir
        kwargs = dict(trace=True, trace_cores=[0])

    res = bass_utils.run_bass_kernel_spmd(
        nc, in_maps, core_ids=list(range(H)), **kwargs)
    LAST_RESULTS = res

    out = res.results[0]["out"].astype(np.float32).copy()
    for h in range(1, H):
        out += res.results[h]["out"]
    return out.reshape(B, T, E)


# revision 10
# speedup vs baseline: 1.0231x; 1.0231x over previous
"""Trainium2 Bass kernel: multi-head attention (B=2, T=2048, E=1024, H=8, D=512),
bias-free QKV/O projections + RoPE + causal softmax.

Sharding: head-parallel across 8 NeuronCores. Core h computes head h fully:
  qT/kT = RoPE(Wq_h @ x.T), v = x @ Wv_h.T         (projection phase)
  scoresT[k,q] = kT.T @ qT   (per 512-wide q tile, causal-skipped k chunks)
  probsT = exp(scale*scoresT + mask)               (no max-subtraction: |s|<=9)
  attnT[d,q] = v.T @ probsT; rowsum accumulated on the Pool engine
  (elementwise adds of exp chunks) + ONE ones-matmul per q tile
  out_h = (attnT/rowsum).T @ Wo_h.T                (partial o_proj, [4096,1024])
Host sums the 8 partial outputs (equivalent to the all-reduce after o_proj).

Matmuls run fp32r except PV + o_proj which run bf16 (probs/v/attn/Wo are
bf16: probs in [0,1] and normalized attn are tolerant; PSUM accumulates fp32).
PSUM is managed as 8 explicitly-tagged banks so phase-to-phase reuse pairs
earliest-freed banks with earliest-needed, keeping the PE stream gap-free.
"""
from contextlib import ExitStack

import numpy as np

B, T, E, H, D = 2, 2048, 1024, 8, 512
NTOK = B * T
SCALE = float(1.0 / np.sqrt(D))
NEG = -1.0e30
ROPE_BASE = 10000.0

PROFILE = False          # set True (e.g. from test.py) to trace core 0
LAST_RESULTS = None      # BassKernelResults of the last run when PROFILE

_CACHE = {}


def _build():
    import concourse.tile as tile
    from concourse import bacc, mybir

    f32 = mybir.dt.float32
    f32r = mybir.dt.float32r
    bf16 = mybir.dt.bfloat16
    AF = mybir.ActivationFunctionType

    nc = bacc.Bacc("TRN2", target_bir_lowering=False, debug=False,
                   enable_asserts=False, num_devices=8)
    xT_d = nc.dram_tensor("xT", [E, NTOK], f32r, kind="ExternalInput").ap()
    wqT_d = nc.dram_tensor("wqT", [E, D], f32r, kind="ExternalInput").ap()
    wkT_d = nc.dram_tensor("wkT", [E, D], f32r, kind="ExternalInput").ap()
    wvT_d = nc.dram_tensor("wvT", [E, D], f32r, kind="ExternalInput").ap()
    woT_d = nc.dram_tensor("woT", [D, E], bf16, kind="ExternalInput").ap()
    cos_d = nc.dram_tensor("cosdt", [D // 2, T], f32, kind="ExternalInput").ap()
    sin_d = nc.dram_tensor("sindt", [D // 2, T], f32, kind="ExternalInput").ap()
    msk_d = nc.dram_tensor("mask4", [4, 128, 512], bf16, kind="ExternalInput").ap()
    out_d = nc.dram_tensor("out", [NTOK, E], f32, kind="ExternalOutput").ap()

    xT_r = xT_d.rearrange("(eo p) t -> p eo t", p=128)     # [128, 8, 4096]
    cos_r = cos_d.rearrange("(fo p) t -> p fo t", p=128)   # [128, 2, 2048]
    sin_r = sin_d.rearrange("(fo p) t -> p fo t", p=128)
    wq_v = wqT_d.rearrange("(eo p) d -> p eo d", p=128)
    wk_v = wkT_d.rearrange("(eo p) d -> p eo d", p=128)
    wv_v = wvT_d.rearrange("(eo p) d -> p eo d", p=128)

    with tile.TileContext(nc) as tc, ExitStack() as top:
        # ---- all pools top-level: no mid-kernel pool boundaries ----
        wp = top.enter_context(tc.tile_pool(name="wp", bufs=1))
        qkvp = top.enter_context(tc.tile_pool(name="qkvp", bufs=1))
        xp = top.enter_context(tc.tile_pool(name="xp", bufs=2))
        csp = top.enter_context(tc.tile_pool(name="csp", bufs=1))
        scrp = top.enter_context(tc.tile_pool(name="scrp", bufs=2))
        epp = top.enter_context(tc.tile_pool(name="epp", bufs=4))
        atp = top.enter_context(tc.tile_pool(name="atp", bufs=1))
        accp = top.enter_context(tc.tile_pool(name="accp", bufs=1))
        ivp = top.enter_context(tc.tile_pool(name="ivp", bufs=1))
        pbp = top.enter_context(tc.tile_pool(name="pbp", bufs=1, space="PSUM"))

        def PB(k):
            return pbp.tile([128, 512], f32, tag=f"B{k}", name=f"B{k}")

        wq_t = wp.tile([128, 8, D], f32r, tag="wq", name="wq")
        wk_t = wp.tile([128, 8, D], f32r, tag="wk", name="wk")
        wv_t = wp.tile([128, 8, D], f32r, tag="wv", name="wv")
        wv = [wv_t[:, e] for e in range(8)]
        mks = wp.tile([128, 4, 512], bf16, tag="mks", name="mks")
        mk = [mks[:, r] for r in range(4)]
        ones = wp.tile([128, 128], f32r, tag="ones", name="ones")
        expre = wp.tile([128, 1], f32, tag="expre", name="expre")
        wo_t = wp.tile([128, 4, E], bf16, tag="wo", name="wo")
        wo = [wo_t[:, d] for d in range(4)]

        qT = [qkvp.tile([128, T], f32r, tag=f"qT{d}", name=f"qT{d}") for d in range(4)]
        kT = [qkvp.tile([128, T], f32r, tag=f"kT{d}", name=f"kT{d}") for d in range(4)]
        vv = [qkvp.tile([128, D], bf16, tag=f"v{t}", name=f"v{t}") for t in range(16)]

        def rope(dstT, i, j, fo, pi, pj, cs, sn, s0):
            c_, s_ = cs[:, fo], sn[:, fo]
            sA = scrp.tile([128, 1024], f32, tag="scr", name="scr")
            sB = scrp.tile([128, 1024], f32, tag="scr", name="scr")
            t0, t1 = sA[:, 0:512], sA[:, 512:1024]
            t2, t3 = sB[:, 0:512], sB[:, 512:1024]
            nc.vector.tensor_mul(t0[:], pi[:], c_)
            nc.vector.tensor_mul(t1[:], pj[:], s_)
            nc.vector.tensor_sub(dstT[i][:, s0:s0 + 512], t0[:], t1[:])
            nc.vector.tensor_mul(t2[:], pi[:], s_)
            nc.vector.tensor_mul(t3[:], pj[:], c_)
            nc.vector.tensor_add(dstT[j][:, s0:s0 + 512], t2[:], t3[:])

        prefetched = {}
        for b in range(B):
            tok0 = b * T

            # ----- projection phase: qT/kT (RoPE'd) and v -----
            for tt in range(4):
                g0 = tok0 + tt * 512
                s0 = tt * 512
                if b == 1 and tt == 0:
                    xt = prefetched["xt"]
                    cs = prefetched["cs"]
                    sn = prefetched["sn"]
                else:
                    xt = xp.tile([128, 8, 512], f32r, tag="xt", name="xt")
                    cs = csp.tile([128, 2, 512], f32, tag="cs", name="cs")
                    sn = csp.tile([128, 2, 512], f32, tag="sn", name="sn")
                if b == 0 and tt == 0:
                    # need-ordered per-chunk DMAs: first matmul only depends
                    # on the e=0 slices; the PE chases the DMA stream.
                    for e in range(8):
                        nc.sync.dma_start(xt[:, e], xT_r[:, e, g0:g0 + 512])
                        nc.sync.dma_start(wv_t[:, e], wv_v[:, e])
                    for e in range(8):
                        nc.sync.dma_start(wq_t[:, e], wq_v[:, e])
                    nc.sync.dma_start(cs[:], cos_r[:, :, s0:s0 + 512])
                    nc.sync.dma_start(sn[:], sin_r[:, :, s0:s0 + 512])
                    for e in range(8):
                        nc.sync.dma_start(wk_t[:, e], wk_v[:, e])
                    # warmup: ramp the PE clock while the first DMAs stream
                    # in (memset cannot target f32r: set f32 then cast-copy)
                    onef = scrp.tile([128, 1024], f32, tag="scr", name="scr")
                    nc.vector.memset(onef[:, :128], 1.0)
                    nc.vector.tensor_copy(ones[:], onef[:, :128])
                    warm_ps = PB(4)
                    for w in range(4):
                        nc.tensor.matmul(warm_ps[:, :128], ones[:], ones[:],
                                         start=(w == 0), stop=(w == 3))
                    # touch Exp so its ACT table loads before the first score
                    nc.scalar.activation(expre[:], warm_ps[:, :1], AF.Exp,
                                         scale=0.001)
                    nc.vector.tensor_copy(expre[:], expre[:])
                elif not (b == 1 and tt == 0):
                    nc.sync.dma_start(xt[:], xT_r[:, :, g0:g0 + 512])
                    nc.sync.dma_start(cs[:], cos_r[:, :, s0:s0 + 512])
                    nc.sync.dma_start(sn[:], sin_r[:, :, s0:s0 + 512])

                def emit_v(tt=tt, xt=xt):
                    for t4 in range(4):
                        ps_t = PB(4 + (t4 % 2))
                        for e in range(8):
                            nc.tensor.matmul(
                                ps_t[:],
                                xt[:, e, t4 * 128:(t4 + 1) * 128],
                                wv[e][:],
                                start=(e == 0), stop=(e == 7))
                        nc.scalar.copy(vv[tt * 4 + t4][:], ps_t[:])

                def emit_qk(s0=s0, xt=xt, cs=cs, sn=sn):
                    rot = (6, 7, 0, 1, 2, 3)
                    ri = 0
                    for w_t, dstT in ((wq_t, qT), (wk_t, kT)):
                        for i, j, fo in ((0, 2, 0), (1, 3, 1)):
                            ps2 = []
                            for dc in (i, j):
                                ps_t = PB(rot[ri % 6])
                                ri += 1
                                for e in range(8):
                                    nc.tensor.matmul(
                                        ps_t[:],
                                        w_t[:, e, dc * 128:(dc + 1) * 128],
                                        xt[:, e],
                                        start=(e == 0), stop=(e == 7))
                                ps2.append(ps_t)
                            rope(dstT, i, j, fo, ps2[0], ps2[1], cs, sn, s0)

                if b == 0 and tt == 0:
                    # e-outer ordering so each matmul only needs one 256KB
                    # slice: the PE starts ~14us earlier and chases the DMAs.
                    vps = [PB(k) for k in range(4)]
                    for e in range(8):
                        for t4 in range(4):
                            nc.tensor.matmul(
                                vps[t4][:],
                                xt[:, e, t4 * 128:(t4 + 1) * 128],
                                wv[e][:],
                                start=(e == 0), stop=(e == 7))
                    for t4 in range(4):
                        nc.scalar.copy(vv[t4][:], vps[t4][:])
                    for w_t, dstT, b0k in ((wq_t, qT, 4), (wk_t, kT, 0)):
                        ps = {dc: PB(b0k + di)
                              for di, dc in enumerate((0, 2, 1, 3))}
                        for e in range(8):
                            for dc in (0, 2, 1, 3):
                                nc.tensor.matmul(
                                    ps[dc][:],
                                    w_t[:, e, dc * 128:(dc + 1) * 128],
                                    xt[:, e],
                                    start=(e == 0), stop=(e == 7))
                        rope(dstT, 0, 2, 0, ps[0], ps[2], cs, sn, s0)
                        rope(dstT, 1, 3, 1, ps[1], ps[3], cs, sn, s0)
                elif tt < 3 and not (b == 1 and tt == 0):
                    # v first (its ACT-copy evacuation has no cos/sin dep)
                    emit_v()
                    emit_qk()
                else:
                    # qk first: on tt3 it ends the phase with a short ACT
                    # tail; on b1/tt0 the qk banks are the ones the previous
                    # batch's attention frees first.
                    emit_qk()
                    emit_v()

            if b == 0:
                # prefetch batch 1's first x tile + rope tables during batch
                # 0's attention so its projection starts with no DMA stall
                xt_pre = xp.tile([128, 8, 512], f32r, tag="xt", name="xt")
                cs_pre = csp.tile([128, 2, 512], f32, tag="cs", name="cs")
                sn_pre = csp.tile([128, 2, 512], f32, tag="sn", name="sn")
                prefetched.update(xt=xt_pre, cs=cs_pre, sn=sn_pre)

            # ----- attention + o_proj phase -----
            SC = (0, 1, 2)
            ATB = (3, 4, 6, 7)
            if b == 0:
                nc.sync.dma_start(mks[:], msk_d.rearrange("r p q -> p r q"))
                nc.sync.dma_start(
                    wo_t[:], woT_d.rearrange("(do p) e -> p do e", p=128))
                nc.sync.dma_start(prefetched["xt"][:], xT_r[:, :, T:T + 512])
                nc.sync.dma_start(prefetched["cs"][:], cos_r[:, :, 0:512])
                nc.sync.dma_start(prefetched["sn"][:], sin_r[:, :, 0:512])

            sci = [0]

            def emit_oproj(n):
                q0 = n * 512
                for t4 in range(4):
                    ob = scrp.tile([128, 1024], f32, tag="scr", name="scr")
                    for et in range(2):
                        op_ps = PB(SC[sci[0] % 3])
                        sci[0] += 1
                        for dc in range(4):
                            nc.tensor.matmul(
                                op_ps[:],
                                at_sb[n % 2][dc][:, t4 * 128:(t4 + 1) * 128],
                                wo[dc][:, et * 512:(et + 1) * 512],
                                start=(dc == 0), stop=(dc == 3))
                        nc.scalar.copy(ob[:, et * 512:(et + 1) * 512], op_ps[:])
                    r0 = tok0 + q0 + t4 * 128
                    nc.sync.dma_start(out_d[r0:r0 + 128, :], ob[:])

            at_sb = {0: None, 1: None}
            for n in range(4):
                q0 = n * 512
                nch = 4 * n + 4
                attn_ps = [PB(ATB[d]) for d in range(4)]
                acc = accp.tile([128, 512], f32r, tag="acc", name="acc")

                def emit_pv(pex, pc, nch=nch, attn_ps=attn_ps):
                    for dc in range(4):
                        nc.tensor.matmul(
                            attn_ps[dc][:],
                            vv[pc][:, dc * 128:(dc + 1) * 128], pex[:],
                            start=(pc == 0), stop=(pc == nch - 1))

                pending = []
                oproj_done = (n == 0)
                for c in range(nch):
                    sc_ps = PB(SC[sci[0] % 3])
                    sci[0] += 1
                    for dc in range(4):
                        nc.tensor.matmul(
                            sc_ps[:],
                            kT[dc][:, c * 128:(c + 1) * 128],
                            qT[dc][:, q0:q0 + 512],
                            start=(dc == 0), stop=(dc == 3))
                    if c >= 4 * n:
                        nc.vector.tensor_add(sc_ps[:], sc_ps[:], mk[c - 4 * n][:])
                    ex = epp.tile([128, 512], bf16, tag="ex", name="ex")
                    nc.scalar.activation(ex[:], sc_ps[:], AF.Exp, scale=SCALE)
                    # rowsum accumulates on the (otherwise idle) Pool engine
                    if c == 0:
                        nc.gpsimd.tensor_copy(acc[:], ex[:])
                    else:
                        nc.gpsimd.tensor_add(acc[:], acc[:], ex[:])
                    pending.append((ex, c))
                    if len(pending) > 3:
                        emit_pv(*pending.pop(0))
                    # deferred so the previous group's normalize (DVE) has
                    # finished by the time the PE reaches these matmuls
                    if c == 5 and not oproj_done:
                        emit_oproj(n - 1)
                        oproj_done = True
                for pex, pc in pending:
                    emit_pv(pex, pc)
                # single partition-reduce matmul per q tile (broadcast rows)
                rs_ps = PB(5)
                nc.tensor.matmul(rs_ps[:], ones[:], acc[:], start=True, stop=True)
                inv = ivp.tile([128, 512], f32, tag="inv", name="inv")
                nc.vector.reciprocal(inv[:], rs_ps[:])
                at_sb[n % 2] = [
                    atp.tile([128, 512], bf16, tag=f"at{n % 2}_{dc}",
                             name=f"at{n % 2}_{dc}")
                    for dc in range(4)]
                if n == 3:
                    # column-sliced so o_proj's first t4 tile unblocks after
                    # 4 small muls instead of the full normalize
                    for t4 in range(4):
                        cl = slice(t4 * 128, (t4 + 1) * 128)
                        for dc in range(4):
                            nc.vector.tensor_mul(
                                at_sb[1][dc][:, cl], attn_ps[dc][:, cl],
                                inv[:, cl])
                else:
                    for dc in range(4):
                        nc.vector.tensor_mul(
                            at_sb[n % 2][dc][:], attn_ps[dc][:], inv[:])
            emit_oproj(3)
    nc.compile()
    return nc


def _host_tables():
    import ml_dtypes
    inv_freq = 1.0 / (ROPE_BASE ** (np.arange(0, D, 2, dtype=np.float64) / D))
    ang = np.arange(T, dtype=np.float64)[:, None] * inv_freq[None, :]  # [T, D/2]
    cosdt = np.ascontiguousarray(np.cos(ang).T.astype(np.float32))     # [D/2, T]
    sindt = np.ascontiguousarray(np.sin(ang).T.astype(np.float32))
    mask4 = np.zeros((4, 128, 512), dtype=np.float32)
    kk = np.arange(128)[:, None]
    qq = np.arange(512)[None, :]
    for r in range(4):
        mask4[r] = np.where(128 * r + kk <= qq, 0.0, NEG).astype(np.float32)
    return cosdt, sindt, mask4.astype(ml_dtypes.bfloat16)


def kernel(x, Wq, Wk, Wv, Wo):
    global LAST_RESULTS
    import ml_dtypes
    from concourse import bass_utils

    if "nc" not in _CACHE:
        _CACHE["nc"] = _build()
    nc = _CACHE["nc"]

    x = np.asarray(x, dtype=np.float32)
    Wq = np.asarray(Wq, dtype=np.float32)
    Wk = np.asarray(Wk, dtype=np.float32)
    Wv = np.asarray(Wv, dtype=np.float32)
    Wo = np.asarray(Wo, dtype=np.float32)

    xT = np.ascontiguousarray(x.reshape(NTOK, E).T)          # [E, NTOK]
    cosdt, sindt, mask4 = _host_tables()

    in_maps = []
    for h in range(H):
        in_maps.append({
            "xT": xT,
            "wqT": np.ascontiguousarray(Wq[h * D:(h + 1) * D, :].T),
            "wkT": np.ascontiguousarray(Wk[h * D:(h + 1) * D, :].T),
            "wvT": np.ascontiguousarray(Wv[h * D:(h + 1) * D, :].T),
            "woT": np.ascontiguousarray(
                Wo[:, h * D:(h + 1) * D].T).astype(ml_dtypes.bfloat16),
            "cosdt": cosdt,
            "sindt": sindt,
            "mask4": mask4,
        })

    kwargs = {}
    if PROFILE:
        import sys
        import types
        import trn_agent_boot.trn_boot as _tb
        hook = _tb._ntff_profile_via_ctypes("/opt/axon/libaxon_pjrt.so")
        mod = types.ModuleType("antenv.axon_hooks")
        mod.get_axon_ntff_profile_hook = lambda: hook
        mod.set_axon_ntff_profile_hook = lambda h_: None
        sys.modules["antenv.axon_hooks"] = mod
        bass_utils.upload_artifacts = lambda tmpdir: tmpdir
        kwargs = dict(trace=True, trace_cores=[0])

    res = bass_utils.run_bass_kernel_spmd(
        nc, in_maps, core_ids=list(range(H)), **kwargs)
    LAST_RESULTS = res

    out = res.results[0]["out"].astype(np.float32).copy()
    for h in range(1, H):
        out += res.results[h]["out"]
    return out.reshape(B, T, E)
